# revision 13
# baseline (speedup 1.0000x reference)
"""DoReFa dense layer (bitW=1, bitA=3) on 8 Trainium2 NeuronCores.

out = quantize_act(clip(|x|,0,1), 3b) @ (sign(W) * mean|W|) + b

Math used by the kernel (exact):
    a_int = round(min(7*|x|, 7))   in {0..7}   -> exact in bf16/fp8
    S     = sign(W)                in {-1,0,1} -> exact in fp8
    out   = (E/7) * (a_int @ S) + b,  E = mean|W| (computed on device)

The integer matmul accumulates exactly in fp32 PSUM (|sums| <= 28672 < 2^15),
so intermediate results are stored as int16 and scaled by E/7 at the end.

Sharding: data-parallel over batch (8 x 1024 rows), W replicated.
"""

import sys

sys.path.insert(0, "/opt/trn_rl_repo")

from contextlib import ExitStack

import numpy as np
from concourse import bacc, mybir, tile
from concourse import bass_isa
from concourse.bass_utils import run_bass_kernel_spmd

# Problem dims (hardcoded per contract)
BATCH, IN_CH, N_UNITS = 8192, 4096, 4096
N_CORES = 8
P = 128

M = BATCH // N_CORES  # 1024 rows per core
KO = IN_CH // P  # 32 k-subtiles of 128
MT = M // P  # 8 m-subtiles of 128
NBS = 512  # n-block width
NB = N_UNITS // NBS  # 8 n-blocks
KC = 4  # k-subtiles per W dma chunk
NCH = KO // KC  # 8 chunks per n-block
KH = 1024  # k columns per activation quant chunk

MAGIC = float(2**23)

F32 = mybir.dt.float32
BF16 = mybir.dt.bfloat16
FP8 = mybir.dt.float8e4
I16 = mybir.dt.int16
AF = mybir.ActivationFunctionType
ALU = mybir.AluOpType


def _body(ctx, tc, x, w, b, out, use_dr):
    nc = tc.nc

    xr = x.rearrange("(mt p) k -> mt p k", p=P)
    wr = w.rearrange("(ko p) n -> p ko n", p=P)
    outr = out.rearrange("(mt p) n -> mt p n", p=P)

    const = ctx.enter_context(tc.tile_pool(name="const", bufs=1))
    xs_pool = ctx.enter_context(tc.tile_pool(name="xs", bufs=2))
    qb_pool = ctx.enter_context(tc.tile_pool(name="qb", bufs=2))
    stg_pool = ctx.enter_context(tc.tile_pool(name="stg", bufs=10))
    ws_pool = ctx.enter_context(tc.tile_pool(name="ws", bufs=2))
    ss_pool = ctx.enter_context(tc.tile_pool(name="ss", bufs=3))
    orow_pool = ctx.enter_context(tc.tile_pool(name="orow", bufs=4))
    psum_pool = ctx.enter_context(tc.tile_pool(name="psum", bufs=8, space="PSUM"))

    # Resident tensors (all fp8 activations: 32KB/partition)
    if use_dr:
        # ko-pair tiles for DoubleRow: [p, 2, M] fp8
        aT = [const.tile([P, 2, M], FP8, name=f"aT{i}") for i in range(KO // 2)]
    else:
        aT = [const.tile([P, M], FP8, name=f"aT{i}") for i in range(KO)]
    unscaled = [const.tile([P, N_UNITS], I16, name=f"uns{m}") for m in range(MT)]
    accW = const.tile([P, NB * NCH], F32, name="accW")
    b_bc = const.tile([P, N_UNITS], F32, name="b_bc")
    sAP = const.tile([P, 1], F32, name="sAP")

    nc.scalar.dma_start(b_bc[0:1, :], b[:])
    nc.gpsimd.partition_broadcast(b_bc[:], b_bc[0:1, :], channels=P)

    # ---- Phase A: quantize activations + transpose ----
    # a_q[m, k] = round(min(7*|x|, 7)) as bf16 (exact small ints), then
    # blocked-transpose to stage[ko][p, m] with logical k = ko*128 + p
    # (matches W's "(ko p) n" layout), then cast to fp8.
    for kh in range(IN_CH // KH):
        stage = {}
        for mt in range(MT):
            xs = xs_pool.tile([P, KH], F32, tag="xs")
            nc.scalar.dma_start(xs[:], xr[mt][:, kh * KH : (kh + 1) * KH])
            nc.scalar.activation(xs[:], xs[:], AF.Abs, scale=7.0)
            nc.vector.tensor_scalar(xs[:], xs[:], 7.0, MAGIC, ALU.min, ALU.add)
            qb = qb_pool.tile([P, KH], BF16, tag="qb")
            nc.scalar.activation(qb[:], xs[:], AF.Copy, bias=-MAGIC)
            for koh in range(KH // P):
                ko = kh * (KH // P) + koh
                if ko not in stage:
                    stage[ko] = stg_pool.tile(
                        [P, M], BF16, tag="stg", name=f"stg{ko}"
                    )
                nc.sync.dma_start_transpose(
                    out=stage[ko][:, mt * P : (mt + 1) * P],
                    in_=qb[:, koh * P : (koh + 1) * P],
                )
        for ko in stage:
            if use_dr:
                nc.vector.tensor_copy(aT[ko // 2][:, ko % 2, :], stage[ko][:])
            else:
                nc.vector.tensor_copy(aT[ko][:], stage[ko][:])

    # ---- Phase B: stream W by column blocks; sign-quantize; matmul ----
    for nb in range(NB):
        psums = [
            psum_pool.tile([P, NBS], F32, tag="ps", name=f"ps{nb}_{m}")
            for m in range(MT)
        ]
        for c in range(NCH):
            wt = ws_pool.tile([P, KC, NBS], F32, tag="ws")
            nc.scalar.dma_start(
                wt[:], wr[:, c * KC : (c + 1) * KC, nb * NBS : (nb + 1) * NBS]
            )
            st = ss_pool.tile([P, KC, NBS], FP8, tag="ss")
            nc.scalar.activation(st[:], wt[:], AF.Sign)
            nc.vector.tensor_reduce(
                accW[:, nb * NCH + c : nb * NCH + c + 1],
                wt[:],
                axis=mybir.AxisListType.XY,
                op=ALU.add,
                apply_absolute_value=True,
            )
            for m in range(MT):
                if use_dr:
                    for kq in range(KC // 2):
                        ko = c * KC + 2 * kq
                        nc.tensor.matmul(
                            psums[m][:],
                            aT[ko // 2][:, :, m * P : (m + 1) * P],
                            st[:, 2 * kq : 2 * kq + 2, :],
                            start=(c == 0 and kq == 0),
                            stop=(c == NCH - 1 and kq == KC // 2 - 1),
                            perf_mode=mybir.MatmulPerfMode.DoubleRow,
                        )
                else:
                    for kq in range(KC):
                        ko = c * KC + kq
                        nc.tensor.matmul(
                            psums[m][:],
                            aT[ko][:, m * P : (m + 1) * P],
                            st[:, kq, :],
                            start=(c == 0 and kq == 0),
                            stop=(c == NCH - 1 and kq == KC - 1),
                        )
        for m in range(MT):
            nc.vector.tensor_copy(
                unscaled[m][:, nb * NBS : (nb + 1) * NBS], psums[m][:]
            )

    # ---- Phase C: E = mean|W|; scale = E/7 ----
    accT = const.tile([P, 1], F32, name="accT")
    nc.vector.tensor_reduce(
        accT[:], accW[:], axis=mybir.AxisListType.X, op=ALU.add
    )
    accB = const.tile([P, 1], F32, name="accB")
    nc.gpsimd.partition_all_reduce(
        accB[:], accT[:], channels=P, reduce_op=bass_isa.ReduceOp.add
    )
    nc.vector.tensor_scalar(
        sAP[:], accB[:], 1.0 / (7.0 * IN_CH * N_UNITS), None, ALU.mult
    )

    # ---- Phase D: out = unscaled * (E/7) + b ----
    for m in range(MT):
        for nb in range(NB):
            sl = slice(nb * NBS, (nb + 1) * NBS)
            orow = orow_pool.tile([P, NBS], F32, tag="orow", name=f"or{m}_{nb}")
            nc.vector.tensor_scalar(
                orow[:], unscaled[m][:, sl], sAP[:], None, ALU.mult
            )
            nc.vector.tensor_tensor(orow[:], orow[:], b_bc[:, sl], ALU.add)
            nc.scalar.dma_start(outr[m][:, sl], orow[:])


def build(use_dr=True):
    nc = bacc.Bacc(
        "TRN2", target_bir_lowering=False, debug=False, num_devices=N_CORES
    )
    x = nc.dram_tensor("inputs", [M, IN_CH], F32, kind="ExternalInput").ap()
    w = nc.dram_tensor("W", [IN_CH, N_UNITS], F32, kind="ExternalInput").ap()
    b = nc.dram_tensor("b", [1, N_UNITS], F32, kind="ExternalInput").ap()
    out = nc.dram_tensor("out", [M, N_UNITS], F32, kind="ExternalOutput").ap()
    with tile.TileContext(nc) as tc, ExitStack() as ctx:
        _body(ctx, tc, x, w, b, out, use_dr)
    nc.compile()
    return nc


_cached_nc = None


def _get_nc():
    global _cached_nc
    if _cached_nc is None:
        _cached_nc = build(use_dr=True)
    return _cached_nc


def run(inputs, W, b, trace=False):
    nc = _get_nc()
    b2 = np.ascontiguousarray(b.reshape(1, -1).astype(np.float32, copy=False))
    Wc = np.ascontiguousarray(W.astype(np.float32, copy=False))
    in_maps = []
    for c in range(N_CORES):
        shard = np.ascontiguousarray(inputs[c * M : (c + 1) * M])
        in_maps.append({"inputs": shard, "W": Wc, "b": b2})
    res = run_bass_kernel_spmd(
        nc, in_maps, core_ids=list(range(N_CORES)), trace=trace
    )
    out = np.concatenate([res.results[c]["out"] for c in range(N_CORES)], axis=0)
    return out, res


def kernel(inputs, W, b):
    out, _ = run(inputs, W, b, trace=False)
    return out


if __name__ == "__main__":
    rng = np.random.default_rng(0)
    x = rng.standard_normal((BATCH, IN_CH), dtype=np.float32)
    W = (rng.standard_normal((IN_CH, N_UNITS)) * 0.1).astype(np.float32)
    b = np.zeros(N_UNITS, dtype=np.float32)
    got = kernel(inputs=x, W=W, b=b)
    E = np.abs(W).mean(dtype=np.float64)
    a = np.round(np.minimum(np.abs(x), 1.0) * 7.0)
    want = (a.astype(np.float64) @ np.sign(W).astype(np.float64)) * (E / 7.0)
    err = np.abs(got - want).max() / np.abs(want).max()
    print("rel err vs numpy ref:", err)


# revision 25
# speedup vs baseline: 1.5489x; 1.5489x over previous
"""DoReFa dense layer (bitW=1, bitA=3) on 8 Trainium2 NeuronCores.

out = quantize_act(clip(|x|,0,1), 3b) @ (sign(W) * mean|W|) + b

Math used by the kernel (exact):
    a_int = round(min(7*|x|, 7))   in {0..7}   -> exact in bf16/fp8
    S     = sign(W)                in {-1,0,1} -> exact in fp8
    out   = (E/7) * (a_int @ S) + b,  E = mean|W| (computed on device)

The integer matmul accumulates exactly in fp32 PSUM (|sums| <= 28672 < 2^15),
so intermediate results are stored as int16 and scaled by E/7 at the end.

Sharding: data-parallel over batch (8 x 1024 rows), W replicated.
"""

import sys

sys.path.insert(0, "/opt/trn_rl_repo")

from contextlib import ExitStack

import numpy as np
from concourse import bacc, mybir, tile
from concourse import bass_isa
from concourse.bass_utils import run_bass_kernel_spmd

# Problem dims (hardcoded per contract)
BATCH, IN_CH, N_UNITS = 8192, 4096, 4096
N_CORES = 8
P = 128

M = BATCH // N_CORES  # 1024 rows per core
KO = IN_CH // P  # 32 k-subtiles of 128
MT = M // P  # 8 m-subtiles of 128
NBS = 512  # n-block width
NB = N_UNITS // NBS  # 8 n-blocks
KC = 4  # k-subtiles per W dma chunk
NCH = KO // KC  # 8 chunks per n-block
KH = 2048  # k columns per activation quant chunk

MAGIC = float(2**23)

F32 = mybir.dt.float32
BF16 = mybir.dt.bfloat16
FP8 = mybir.dt.float8e4
I16 = mybir.dt.int16
AF = mybir.ActivationFunctionType
ALU = mybir.AluOpType


def _body(ctx, tc, x, w, b, out, use_dr, add_bias):
    nc = tc.nc

    xr = x.rearrange("(mt p) k -> mt p k", p=P)
    wr = w.rearrange("(ko p) n -> p ko n", p=P)
    outr = out.rearrange("(mt p) n -> mt p n", p=P)

    const = ctx.enter_context(tc.tile_pool(name="const", bufs=1))
    xs_pool = ctx.enter_context(tc.tile_pool(name="xs", bufs=2))
    qb_pool = ctx.enter_context(tc.tile_pool(name="qb", bufs=2))
    stg_pool = ctx.enter_context(tc.tile_pool(name="stg", bufs=4))
    ws_pool = ctx.enter_context(tc.tile_pool(name="ws", bufs=2))
    ss_pool = ctx.enter_context(tc.tile_pool(name="ss", bufs=12))
    orow_pool = ctx.enter_context(tc.tile_pool(name="orow", bufs=4))
    psum_pool = ctx.enter_context(tc.tile_pool(name="psum", bufs=8, space="PSUM"))
    dram_pool = ctx.enter_context(tc.tile_pool(name="dram", bufs=1, space="DRAM"))

    # Resident tensors (all fp8 activations: 32KB/partition)
    if use_dr:
        # ko-pair tiles for DoubleRow: [p, 2, M] fp8
        aT = [const.tile([P, 2, M], FP8, name=f"aT{i}") for i in range(KO // 2)]
    else:
        aT = [const.tile([P, M], FP8, name=f"aT{i}") for i in range(KO)]
    unscaled = [const.tile([P, N_UNITS], I16, name=f"uns{m}") for m in range(MT)]
    accW = const.tile([P, NB * NCH], F32, name="accW")
    sAP = const.tile([P, 1], F32, name="sAP")

    if add_bias:
        b_bc = const.tile([P, N_UNITS], F32, name="b_bc")
        nc.scalar.dma_start(b_bc[0:1, :], b[:])
        nc.gpsimd.partition_broadcast(b_bc[:], b_bc[0:1, :], channels=P)

    # ---- Phase A: quantize activations + transpose ----
    # a_q[m, k] = round(min(7*|x|, 7)) as bf16 (exact small ints) written to
    # a DRAM scratch, then one xbar DMA-transpose per 128-col chunk gives
    # stage[p, m] with logical k = ko*128 + p (matches W's "(ko p) n"
    # layout), cast to fp8 into the resident aT.
    aq_dram = dram_pool.tile([M, IN_CH], BF16, name="aq_dram")
    for kh in range(IN_CH // KH):
        for mt in range(MT):
            xs = xs_pool.tile([P, KH], F32, tag="xs")
            nc.scalar.dma_start(xs[:], xr[mt][:, kh * KH : (kh + 1) * KH])
            nc.scalar.activation(xs[:], xs[:], AF.Abs, scale=7.0)
            nc.vector.tensor_scalar(xs[:], xs[:], 7.0, MAGIC, ALU.min, ALU.add)
            qb = qb_pool.tile([P, KH], BF16, tag="qb")
            nc.scalar.activation(qb[:], xs[:], AF.Copy, bias=-MAGIC)
            nc.scalar.dma_start(
                aq_dram[mt * P : (mt + 1) * P, kh * KH : (kh + 1) * KH], qb[:]
            )
        for koh in range(KH // P):
            ko = kh * (KH // P) + koh
            stg = stg_pool.tile([P, M], BF16, tag="stg", name=f"stg{ko}")
            nc.sync.dma_start_transpose(
                out=stg[:], in_=aq_dram[:, ko * P : (ko + 1) * P]
            )
            if use_dr:
                nc.vector.tensor_copy(aT[ko // 2][:, ko % 2, :], stg[:])
            else:
                nc.vector.tensor_copy(aT[ko][:], stg[:])

    # ---- Phase B: stream W by column blocks; sign-quantize; matmul ----
    for nb in range(NB):
        psums = [
            psum_pool.tile([P, NBS], F32, tag="ps", name=f"ps{nb}_{m}")
            for m in range(MT)
        ]
        for c in range(NCH):
            wt = ws_pool.tile([P, KC, NBS], F32, tag="ws")
            nc.scalar.dma_start(
                wt[:], wr[:, c * KC : (c + 1) * KC, nb * NBS : (nb + 1) * NBS]
            )
            st = ss_pool.tile([P, KC, NBS], FP8, tag="ss")
            # S' = (W>=0) - 0.5 in {+-0.5}; matmul result is then M'/2,
            # compensated by scaling outputs with 2E/7 instead of E/7.
            nc.vector.tensor_scalar(
                st[:], wt[:], 0.0, 0.5, ALU.is_ge, ALU.subtract
            )
            # |W| in-place on ACT with fused free-dim sum into accW column
            nc.scalar.activation(
                wt[:],
                wt[:],
                AF.Abs,
                accum_out=accW[:, nb * NCH + c : nb * NCH + c + 1],
            )
            for m in range(MT):
                if use_dr:
                    for kq in range(KC // 2):
                        ko = c * KC + 2 * kq
                        nc.tensor.matmul(
                            psums[m][:],
                            aT[ko // 2][:, :, m * P : (m + 1) * P],
                            st[:, 2 * kq : 2 * kq + 2, :],
                            start=(c == 0 and kq == 0),
                            stop=(c == NCH - 1 and kq == KC // 2 - 1),
                            perf_mode=mybir.MatmulPerfMode.DoubleRow,
                        )
                else:
                    for kq in range(KC):
                        ko = c * KC + kq
                        nc.tensor.matmul(
                            psums[m][:],
                            aT[ko][:, m * P : (m + 1) * P],
                            st[:, kq, :],
                            start=(c == 0 and kq == 0),
                            stop=(c == NCH - 1 and kq == KC - 1),
                        )
        for m in range(MT):
            # psum holds M'/2 (half-integers when rowsum(a) is odd);
            # double to exact integers before the int16 store.
            nc.vector.tensor_scalar(
                unscaled[m][:, nb * NBS : (nb + 1) * NBS],
                psums[m][:],
                2.0,
                None,
                ALU.mult,
            )

    # ---- Phase C: E = mean|W|; scale = E/7 ----
    accT = const.tile([P, 1], F32, name="accT")
    nc.vector.tensor_reduce(
        accT[:], accW[:], axis=mybir.AxisListType.X, op=ALU.add
    )
    accB = const.tile([P, 1], F32, name="accB")
    nc.gpsimd.partition_all_reduce(
        accB[:], accT[:], channels=P, reduce_op=bass_isa.ReduceOp.add
    )
    nc.vector.tensor_scalar(
        sAP[:], accB[:], 1.0 / (7.0 * IN_CH * N_UNITS), None, ALU.mult
    )

    # ---- Phase D: out = unscaled * (E/7) + b ----
    for m in range(MT):
        for nb in range(NB):
            sl = slice(nb * NBS, (nb + 1) * NBS)
            orow = orow_pool.tile([P, NBS], F32, tag="orow", name=f"or{m}_{nb}")
            nc.vector.tensor_scalar(
                orow[:], unscaled[m][:, sl], sAP[:], None, ALU.mult
            )
            if add_bias:
                nc.vector.tensor_tensor(
                    orow[:], orow[:], b_bc[:, sl], ALU.add
                )
            nc.scalar.dma_start(outr[m][:, sl], orow[:])


def build(use_dr=True, add_bias=True):
    nc = bacc.Bacc(
        "TRN2", target_bir_lowering=False, debug=False, num_devices=N_CORES
    )
    x = nc.dram_tensor("inputs", [M, IN_CH], F32, kind="ExternalInput").ap()
    w = nc.dram_tensor("W", [IN_CH, N_UNITS], F32, kind="ExternalInput").ap()
    b = nc.dram_tensor("b", [1, N_UNITS], F32, kind="ExternalInput").ap()
    out = nc.dram_tensor("out", [M, N_UNITS], F32, kind="ExternalOutput").ap()
    with tile.TileContext(nc) as tc, ExitStack() as ctx:
        _body(ctx, tc, x, w, b, out, use_dr, add_bias)
    nc.compile()
    return nc


_cached = {}


def _get_nc(add_bias):
    key = add_bias
    if key not in _cached:
        _cached[key] = build(use_dr=True, add_bias=add_bias)
    return _cached[key]


def _expected_inputs(nc):
    import concourse.mybir as mb

    names = set()
    for alloc in nc.m.functions[0].allocations:
        if isinstance(alloc, mb.MemoryLocationSet) and alloc.kind == "ExternalInput":
            names.add(alloc.memorylocations[0].name)
    return names


def run(inputs, W, b, trace=False):
    add_bias = bool(np.any(b))
    nc = _get_nc(add_bias)
    want = _expected_inputs(nc)
    b2 = np.ascontiguousarray(b.reshape(1, -1).astype(np.float32, copy=False))
    Wc = np.ascontiguousarray(W.astype(np.float32, copy=False))
    in_maps = []
    for c in range(N_CORES):
        shard = np.ascontiguousarray(inputs[c * M : (c + 1) * M])
        full = {"inputs": shard, "W": Wc, "b": b2}
        in_maps.append({k: v for k, v in full.items() if k in want})
    res = run_bass_kernel_spmd(
        nc, in_maps, core_ids=list(range(N_CORES)), trace=trace
    )
    out = np.concatenate([res.results[c]["out"] for c in range(N_CORES)], axis=0)
    return out, res


def kernel(inputs, W, b):
    out, _ = run(inputs, W, b, trace=False)
    return out


if __name__ == "__main__":
    rng = np.random.default_rng(0)
    x = rng.standard_normal((BATCH, IN_CH), dtype=np.float32)
    W = (rng.standard_normal((IN_CH, N_UNITS)) * 0.1).astype(np.float32)
    b = np.zeros(N_UNITS, dtype=np.float32)
    got = kernel(inputs=x, W=W, b=b)
    E = np.abs(W).mean(dtype=np.float64)
    a = np.round(np.minimum(np.abs(x), 1.0) * 7.0)
    want = (a.astype(np.float64) @ np.sign(W).astype(np.float64)) * (E / 7.0)
    err = np.abs(got - want).max() / np.abs(want).max()
    print("rel err vs numpy ref:", err)


# revision 29
# speedup vs baseline: 1.8118x; 1.1697x over previous
"""DoReFa dense layer (bitW=1, bitA=3) on 8 Trainium2 NeuronCores.

out = quantize_act(clip(|x|,0,1), 3b) @ (sign(W) * mean|W|) + b

Math used by the kernel (exact):
    a_int = round(min(7*|x|, 7))   in {0..7}   -> exact in bf16/fp8
    S     = sign(W)                in {-1,0,1} -> exact in fp8
    out   = (E/7) * (a_int @ S) + b,  E = mean|W| (computed on device)

The integer matmul accumulates exactly in fp32 PSUM (|sums| <= 28672 < 2^15),
so intermediate results are stored as int16 and scaled by E/7 at the end.

Sharding: data-parallel over batch (8 x 1024 rows), W replicated.
"""

import sys

sys.path.insert(0, "/opt/trn_rl_repo")

from contextlib import ExitStack

import numpy as np
from concourse import bacc, mybir, tile
from concourse import bass_isa
from concourse.bass_utils import run_bass_kernel_spmd

# Problem dims (hardcoded per contract)
BATCH, IN_CH, N_UNITS = 8192, 4096, 4096
N_CORES = 8
P = 128

M = BATCH // N_CORES  # 1024 rows per core
KO = IN_CH // P  # 32 k-subtiles of 128
MT = M // P  # 8 m-subtiles of 128
NBS = 512  # n-block width
NB = N_UNITS // NBS  # 8 n-blocks
KC = 4  # k-subtiles per W dma chunk
NCH = KO // KC  # 8 chunks per n-block
KH = 2048  # k columns per activation quant chunk

MAGIC = float(2**23)

F32 = mybir.dt.float32
BF16 = mybir.dt.bfloat16
FP8 = mybir.dt.float8e4
I16 = mybir.dt.int16
AF = mybir.ActivationFunctionType
ALU = mybir.AluOpType


def _body(ctx, tc, x, w, b, out, use_dr, add_bias):
    nc = tc.nc

    xr = x.rearrange("(mt p) k -> mt p k", p=P)
    # row = kc*256 + 2p + t: partition p holds the adjacent row pair
    # (2p, 2p+1) of each 256-row group kc -- matches the aT u16 pairing.
    wr = w.rearrange("(kc p two) n -> p kc two n", p=P, two=2)
    outr = out.rearrange("(mt p) n -> mt p n", p=P)

    const = ctx.enter_context(tc.tile_pool(name="const", bufs=1))
    xs_pool = ctx.enter_context(tc.tile_pool(name="xs", bufs=2))
    qb_pool = ctx.enter_context(tc.tile_pool(name="qb", bufs=2))
    stg_pool = ctx.enter_context(tc.tile_pool(name="stg", bufs=4))
    ws_pool = ctx.enter_context(tc.tile_pool(name="ws", bufs=2))
    ss_pool = ctx.enter_context(tc.tile_pool(name="ss", bufs=12))
    orow_pool = ctx.enter_context(tc.tile_pool(name="orow", bufs=4))
    psum_pool = ctx.enter_context(tc.tile_pool(name="psum", bufs=8, space="PSUM"))
    dram_pool = ctx.enter_context(tc.tile_pool(name="dram", bufs=1, space="DRAM"))

    # Resident tensors (all fp8 activations: 32KB/partition)
    if use_dr:
        # ko-pair tiles for DoubleRow: [p, 2, M] fp8
        aT = [const.tile([P, 2, M], FP8, name=f"aT{i}") for i in range(KO // 2)]
    else:
        aT = [const.tile([P, M], FP8, name=f"aT{i}") for i in range(KO)]
    unscaled = [const.tile([P, N_UNITS], I16, name=f"uns{m}") for m in range(MT)]
    accW = const.tile([P, NB * NCH], F32, name="accW")
    sAP = const.tile([P, 1], F32, name="sAP")

    if add_bias:
        b_bc = const.tile([P, N_UNITS], F32, name="b_bc")
        nc.scalar.dma_start(b_bc[0:1, :], b[:])
        nc.gpsimd.partition_broadcast(b_bc[:], b_bc[0:1, :], channels=P)

    # ---- Phase A: quantize activations + transpose ----
    # a_q[m, k] = round(min(7*|x|, 7)) as fp8 (exact small ints) written to
    # a DRAM scratch. The scratch is viewed as u16 (pairs of adjacent k)
    # and xbar-transposed: u16 chunk kc gives stage[p, m] holding k-pair
    # (2*(kc*128+p), +1). A strided DVE copy de-interleaves into the
    # resident aT with DR pairing (even k at parity 0, odd at parity 1).
    aq_dram = dram_pool.tile([M, IN_CH], FP8, name="aq_dram")
    aq_u16 = aq_dram[:].bitcast(mybir.dt.uint16)
    for kh in range(IN_CH // KH):
        for mt in range(MT):
            xs = xs_pool.tile([P, KH], F32, tag="xs")
            nc.sync.dma_start(xs[:], xr[mt][:, kh * KH : (kh + 1) * KH])
            nc.scalar.activation(xs[:], xs[:], AF.Abs, scale=7.0)
            nc.vector.tensor_scalar(xs[:], xs[:], 7.0, MAGIC, ALU.min, ALU.add)
            qb = qb_pool.tile([P, KH], FP8, tag="qb")
            nc.scalar.activation(qb[:], xs[:], AF.Copy, bias=-MAGIC)
            nc.sync.dma_start(
                aq_dram[mt * P : (mt + 1) * P, kh * KH : (kh + 1) * KH], qb[:]
            )
        for kch in range(KH // 256):
            # u16 columns kc*128..+127 <-> fp8 k = kh*KH + kch*256 ...
            kc = (kh * KH) // 256 + kch
            stg = stg_pool.tile([P, M], mybir.dt.uint16, tag="stg", name=f"stg{kc}")
            nc.sync.dma_start_transpose(
                out=stg[:], in_=aq_u16[:, kc * P : (kc + 1) * P]
            )
            stg8 = stg[:].bitcast(FP8).rearrange("p (m two) -> p m two", two=2)
            # partition p holds k0 = 2*(kc*128+p) at byte 0, k0+1 at byte 1.
            # DR pair tile index for k0: (kc*128+p) -> ko_pair kc with the
            # pairing (k even, k odd) == aT[kc][:, 0/1, :].
            if use_dr:
                nc.vector.tensor_copy(aT[kc][:, 0, :], stg8[:, :, 0])
                nc.vector.tensor_copy(aT[kc][:, 1, :], stg8[:, :, 1])
            else:
                nc.vector.tensor_copy(aT[2 * kc][:], stg8[:, :, 0])
                nc.vector.tensor_copy(aT[2 * kc + 1][:], stg8[:, :, 1])

    # ---- Phase B: stream W by column blocks; sign-quantize; matmul ----
    KCP = KC // 2  # kc pair-tiles per W chunk
    for nb in range(NB):
        psums = [
            psum_pool.tile([P, NBS], F32, tag="ps", name=f"ps{nb}_{m}")
            for m in range(MT)
        ]
        for c in range(NCH):
            wt = ws_pool.tile([P, KCP, 2, NBS], F32, tag="ws")
            for j in range(KCP):
                nc.scalar.dma_start(
                    wt[:, j],
                    wr[:, c * KCP + j, :, nb * NBS : (nb + 1) * NBS],
                )
            st = ss_pool.tile([P, KCP, 2, NBS], FP8, tag="ss")
            # S' = (W>=0) - 0.5 in {+-0.5}; matmul result is then M'/2,
            # doubled at psum eviction and scaled by E/7 at the end.
            nc.vector.tensor_scalar(
                st[:], wt[:], 0.0, 0.5, ALU.is_ge, ALU.subtract
            )
            # |W| in-place on ACT with fused free-dim sum into accW column
            nc.scalar.activation(
                wt[:],
                wt[:],
                AF.Abs,
                accum_out=accW[:, nb * NCH + c : nb * NCH + c + 1],
            )
            for m in range(MT):
                if use_dr:
                    for j in range(KCP):
                        nc.tensor.matmul(
                            psums[m][:],
                            aT[c * KCP + j][:, :, m * P : (m + 1) * P],
                            st[:, j, :, :],
                            start=(c == 0 and j == 0),
                            stop=(c == NCH - 1 and j == KCP - 1),
                            perf_mode=mybir.MatmulPerfMode.DoubleRow,
                        )
                else:
                    for j in range(KCP):
                        for t in range(2):
                            nc.tensor.matmul(
                                psums[m][:],
                                aT[2 * (c * KCP + j) + t][:, m * P : (m + 1) * P],
                                st[:, j, t, :],
                                start=(c == 0 and j == 0 and t == 0),
                                stop=(
                                    c == NCH - 1 and j == KCP - 1 and t == 1
                                ),
                            )
        for m in range(MT):
            # psum holds M'/2 (half-integers when rowsum(a) is odd);
            # double to exact integers before the int16 store.
            nc.vector.tensor_scalar(
                unscaled[m][:, nb * NBS : (nb + 1) * NBS],
                psums[m][:],
                2.0,
                None,
                ALU.mult,
            )

    # ---- Phase C: E = mean|W|; scale = E/7 ----
    accT = const.tile([P, 1], F32, name="accT")
    nc.vector.tensor_reduce(
        accT[:], accW[:], axis=mybir.AxisListType.X, op=ALU.add
    )
    accB = const.tile([P, 1], F32, name="accB")
    nc.gpsimd.partition_all_reduce(
        accB[:], accT[:], channels=P, reduce_op=bass_isa.ReduceOp.add
    )
    nc.vector.tensor_scalar(
        sAP[:], accB[:], 1.0 / (7.0 * IN_CH * N_UNITS), None, ALU.mult
    )

    # ---- Phase D: out = unscaled * (E/7) + b ----
    for m in range(MT):
        for nb in range(NB):
            sl = slice(nb * NBS, (nb + 1) * NBS)
            orow = orow_pool.tile([P, NBS], F32, tag="orow", name=f"or{m}_{nb}")
            nc.vector.tensor_scalar(
                orow[:], unscaled[m][:, sl], sAP[:], None, ALU.mult
            )
            if add_bias:
                nc.vector.tensor_tensor(
                    orow[:], orow[:], b_bc[:, sl], ALU.add
                )
            nc.scalar.dma_start(outr[m][:, sl], orow[:])


def build(use_dr=True, add_bias=True):
    nc = bacc.Bacc(
        "TRN2", target_bir_lowering=False, debug=False, num_devices=N_CORES
    )
    x = nc.dram_tensor("inputs", [M, IN_CH], F32, kind="ExternalInput").ap()
    w = nc.dram_tensor("W", [IN_CH, N_UNITS], F32, kind="ExternalInput").ap()
    b = nc.dram_tensor("b", [1, N_UNITS], F32, kind="ExternalInput").ap()
    out = nc.dram_tensor("out", [M, N_UNITS], F32, kind="ExternalOutput").ap()
    with tile.TileContext(nc) as tc, ExitStack() as ctx:
        _body(ctx, tc, x, w, b, out, use_dr, add_bias)
    nc.compile()
    return nc


_cached = {}


def _get_nc(add_bias):
    key = add_bias
    if key not in _cached:
        _cached[key] = build(use_dr=True, add_bias=add_bias)
    return _cached[key]


def _expected_inputs(nc):
    import concourse.mybir as mb

    names = set()
    for alloc in nc.m.functions[0].allocations:
        if isinstance(alloc, mb.MemoryLocationSet) and alloc.kind == "ExternalInput":
            names.add(alloc.memorylocations[0].name)
    return names


def run(inputs, W, b, trace=False):
    add_bias = bool(np.any(b))
    nc = _get_nc(add_bias)
    want = _expected_inputs(nc)
    b2 = np.ascontiguousarray(b.reshape(1, -1).astype(np.float32, copy=False))
    Wc = np.ascontiguousarray(W.astype(np.float32, copy=False))
    in_maps = []
    for c in range(N_CORES):
        shard = np.ascontiguousarray(inputs[c * M : (c + 1) * M])
        full = {"inputs": shard, "W": Wc, "b": b2}
        in_maps.append({k: v for k, v in full.items() if k in want})
    res = run_bass_kernel_spmd(
        nc, in_maps, core_ids=list(range(N_CORES)), trace=trace
    )
    out = np.concatenate([res.results[c]["out"] for c in range(N_CORES)], axis=0)
    return out, res


def kernel(inputs, W, b):
    out, _ = run(inputs, W, b, trace=False)
    return out


if __name__ == "__main__":
    rng = np.random.default_rng(0)
    x = rng.standard_normal((BATCH, IN_CH), dtype=np.float32)
    W = (rng.standard_normal((IN_CH, N_UNITS)) * 0.1).astype(np.float32)
    b = np.zeros(N_UNITS, dtype=np.float32)
    got = kernel(inputs=x, W=W, b=b)
    E = np.abs(W).mean(dtype=np.float64)
    a = np.round(np.minimum(np.abs(x), 1.0) * 7.0)
    want = (a.astype(np.float64) @ np.sign(W).astype(np.float64)) * (E / 7.0)
    err = np.abs(got - want).max() / np.abs(want).max()
    print("rel err vs numpy ref:", err)
